# revision 5
# baseline (speedup 1.0000x reference)
"""GCN layer (gather + scatter-add message passing) on 8 Trainium2 NeuronCores.

Strategy (dst-partitioned node sharding, per the sharding hint):
  - Node blocks of 128; block b is owned by core b % 8, slot b // 8.
  - Host sorts edges by (dst block, src-table-half), appends self-loops, pads
    each (block, half) edge group to a multiple of 128, and precomputes the
    symmetric-normalization weights dinv = 1/sqrt(deg+1) from the integer
    degree counts (pure edge_index-derived metadata, like CSR setup).
  - Device: per (supergroup of 7 slots, table-half), one gpsimd dma_gather
    pulls the raw fp16 x rows for all edges (256B rows, software DGE).  Per
    128-edge tile the vector engine builds the scaled one-hot selection
    matrix S[e, n] = (dstloc[e] == n) * dinv_src[e] with a single fused
    tensor_scalar (is_equal + mult with two per-partition scalars - 4x DVE
    mode), and the tensor engine accumulates psum[f, n] += m[e, f]^T @ S[e, n].
    The dst-sorted layout means each 128-node block accumulates entirely in
    PSUM - no scatter to HBM at all.
  - Per block: out[n, o] = dinv[n] * (agg^T @ W)[n, o] + b[o], DMA'd straight
    to the core's output slice.  (The linear layer commutes with the
    aggregation, so the GEMM runs on the 6.3k aggregated rows per core
    instead of all 50k input rows.)

The edge tables are padded so the instruction stream is identical on all 8
cores (run_bass_kernel_spmd compiles one program); only tensor data differs.
"""

import sys

sys.path.insert(0, "/opt/trn_rl_repo")

import numpy as np

import concourse.bass as bass
import concourse.bacc as bacc
import concourse.mybir as mybir
import concourse.tile as tile
import concourse.tile_sem_assignment as _tsa
from concourse.tile import add_dep_helper

# Tile round-robins SWDGE DMAs over the 8 DMASW sem lanes in scheduling
# order, which lets one sem serve instructions on different SWDGE queues.
# The ucode's per-queue ring reclaim then sees foreign increments (CoreSim
# flags this as "sem locked to SWDGE queue").  Pin lanes per queue instead:
# queue q only ever uses lanes {q, q+4}.
if not getattr(_tsa.TileClockTick, "_gcn_queue_aware", False):
    _orig_assign_tick = _tsa.TileClockTick._assign_tick

    def _assign_tick_queue_aware(self, inst):
        if (
            isinstance(inst, _tsa.DMAInst)
            and inst.engine == mybir.EngineType.Pool
            and not isinstance(inst, _tsa.bass_isa.UserSyncedRemoteDMADescs)
            and self.swdge_sem_count == _tsa.NUM_SWDGE_GLOBAL_SEMS
        ):
            q = getattr(inst, "queue_num", 0) or 0
            toggles = getattr(self, "_gcn_q_toggle", None)
            if toggles is None:
                toggles = self._gcn_q_toggle = [0, 0, 0, 0]
            self.next_sw_dma_idx = q + 4 * toggles[q]
            toggles[q] ^= 1
        return _orig_assign_tick(self, inst)

    _tsa.TileClockTick._assign_tick = _assign_tick_queue_aware
    _tsa.TileClockTick._gcn_queue_aware = True

N = 50000
E = 800000
F = 128          # in/out channels
P = 128
NCORES = 8
NB = 392         # node blocks incl. padding (= 8 * 49)
G = NB // NCORES  # 49 slots per core
LO = 32768       # gather-table split (int16 index limit)
NPAD = 51200     # padded node rows
SPQ = 7          # slots per gather supergroup
NSG = G // SPQ   # 7 supergroups

f32 = mybir.dt.float32
fp16 = mybir.dt.float16
i32 = mybir.dt.int32
i16 = mybir.dt.int16


def _host_prep(x, W, b, edge_index):
    """Index manipulation + edge-metadata staging (no FP math on x/W)."""
    x = np.asarray(x, dtype=np.float32)
    W = np.asarray(W, dtype=np.float32)
    b = np.asarray(b, dtype=np.float32)
    ei = np.asarray(edge_index)
    src = ei[0].astype(np.int64)
    dst = ei[1].astype(np.int64)

    cnt = np.bincount(dst, minlength=NPAD).astype(np.int64)
    dinv = np.zeros(NPAD, np.float32)
    dinv[:N] = 1.0 / np.sqrt(cnt[:N].astype(np.float64) + 1.0)

    # Sort edges by (dst block, src table half).
    ishi = (src >= LO).astype(np.int64)
    blk = dst >> 7
    order = np.lexsort((ishi, blk))
    src_s, dst_s, ishi_s = src[order], dst[order], ishi[order]
    blk_s = blk[order]
    bounds = np.searchsorted(blk_s, np.arange(NB + 1))

    # Per (core, slot) edge lists.  block = 8*g + c.
    lo_idx = [[None] * G for _ in range(NCORES)]
    lo_dst = [[None] * G for _ in range(NCORES)]
    hi_idx = [[None] * G for _ in range(NCORES)]
    hi_dst = [[None] * G for _ in range(NCORES)]
    for g in range(G):
        for c in range(NCORES):
            bb_ = 8 * g + c
            s0, s1 = bounds[bb_], bounds[bb_ + 1]
            mid = s0 + int(np.searchsorted(ishi_s[s0:s1], 1))
            sl = np.arange(128 * bb_, min(128 * (bb_ + 1), N), dtype=np.int64)
            li, ld = src_s[s0:mid], dst_s[s0:mid] - 128 * bb_
            hi, hd = src_s[mid:s1] - LO, dst_s[mid:s1] - 128 * bb_
            if bb_ < LO // 128:  # self loops go in the lo half
                li = np.concatenate([li, sl])
                ld = np.concatenate([ld, sl - 128 * bb_])
            else:
                hi = np.concatenate([hi, sl - LO])
                hd = np.concatenate([hd, sl - 128 * bb_])
            lo_idx[c][g], lo_dst[c][g] = li, ld
            hi_idx[c][g], hi_dst[c][g] = hi, hd

    # Shared tile counts (max over cores) keep the instruction stream uniform.
    T_LO = [max(1, max(-(-len(lo_idx[c][g]) // 128) for c in range(NCORES)))
            for g in range(G)]
    T_HI = [max(-(-len(hi_idx[c][g]) // 128) for c in range(NCORES))
            for g in range(G)]
    NT = sum(T_LO) + sum(T_HI)
    LTOT = NT * 8

    x_pad = np.zeros((NPAD, F), np.float16)
    x_pad[:N] = x.astype(np.float16)
    bb_host = np.tile(b[None, :], (P, 1)).astype(np.float32)
    iota_host = np.tile(np.arange(P, dtype=np.float16)[None, :], (P, 1)).copy()

    in_maps = []
    for c in range(NCORES):
        dstloc = np.full((P, NT), -1.0, np.float16)
        dinv_e = np.zeros((P, NT), np.float32)  # dinv[src] per edge
        idx16 = np.zeros((P, LTOT), np.int16)
        # Packing order mirrors the device issue order: per supergroup,
        # lo halves of its 7 slots, then hi halves of its 7 slots.
        col = icol = 0
        for sg in range(NSG):
            for side in ("lo", "hi"):
                for g in range(SPQ * sg, SPQ * (sg + 1)):
                    if side == "lo":
                        nt, li, ld = T_LO[g], lo_idx[c][g], lo_dst[c][g]
                        di = dinv[li]
                    else:
                        nt, li, ld = T_HI[g], hi_idx[c][g], hi_dst[c][g]
                        di = dinv[li + LO]
                    if nt == 0:
                        continue
                    pi = np.zeros(nt * 128, np.int64)
                    pi[: len(li)] = li
                    pd = np.full(nt * 128, -1.0, np.float32)
                    pd[: len(ld)] = ld
                    pv = np.zeros(nt * 128, np.float32)
                    pv[: len(li)] = di
                    dstloc[:, col : col + nt] = pd.reshape(nt, 128).T
                    dinv_e[:, col : col + nt] = pv.reshape(nt, 128).T
                    col += nt
                    k8 = nt * 8
                    idx16[:, icol : icol + k8] = np.tile(
                        pi.reshape(-1, 16).T.astype(np.int16), (8, 1)
                    )
                    icol += k8
        # dinv per owned node, laid out [128 node-in-block, G slots]
        dinv_slot = dinv.reshape(NPAD // P, P).T[:, [8 * g + c for g in range(G)]]
        in_maps.append(
            {
                "x": x_pad,
                "w": W,
                "bb": bb_host,
                "iota": iota_host,
                "dstloc": dstloc,
                "dinv_e": dinv_e,
                "dinv_slot": np.ascontiguousarray(dinv_slot, np.float32),
                "idx16": idx16,
            }
        )
    return in_maps, T_LO, T_HI


def build_nc(T_LO, T_HI, debug=False):
    NT = sum(T_LO) + sum(T_HI)
    LTOT = NT * 8
    nc = bacc.Bacc(
        "TRN2", target_bir_lowering=False, debug=debug, num_swdge_queues=4
    )

    x_d = nc.dram_tensor("x", [NPAD, F], fp16, kind="ExternalInput")
    w_d = nc.dram_tensor("w", [F, F], f32, kind="ExternalInput")
    bb_d = nc.dram_tensor("bb", [P, F], f32, kind="ExternalInput")
    iota_d = nc.dram_tensor("iota", [P, P], fp16, kind="ExternalInput")
    dstloc_d = nc.dram_tensor("dstloc", [P, NT], fp16, kind="ExternalInput")
    dinv_e_d = nc.dram_tensor("dinv_e", [P, NT], f32, kind="ExternalInput")
    dinv_slot_d = nc.dram_tensor("dinv_slot", [P, G], f32, kind="ExternalInput")
    idx_d = nc.dram_tensor("idx16", [P, LTOT], i16, kind="ExternalInput")
    out_d = nc.dram_tensor("out", [G * P, F], f32, kind="ExternalOutput")

    # Per-(supergroup, half) tile counts and column offsets.
    sg_lo = [sum(T_LO[SPQ * s : SPQ * (s + 1)]) for s in range(NSG)]
    sg_hi = [sum(T_HI[SPQ * s : SPQ * (s + 1)]) for s in range(NSG)]

    with tile.TileContext(nc) as tc:
        with (
            tc.tile_pool(name="const", bufs=1) as cp,
            tc.tile_pool(name="mlo", bufs=2) as plo,
            tc.tile_pool(name="mhi", bufs=2) as phi,
            tc.tile_pool(name="sel", bufs=6) as psel,
            tc.tile_pool(name="tt", bufs=3) as ptt,
            tc.tile_pool(name="osb", bufs=3) as posb,
            tc.tile_pool(name="agg", bufs=4, space="PSUM") as pagg,
            tc.tile_pool(name="gem", bufs=2, space="PSUM") as pgem,
        ):
            w_sb = cp.tile([F, F], f32)
            nc.sync.dma_start(out=w_sb[:], in_=w_d[:])
            bb_sb = cp.tile([P, F], f32)
            nc.sync.dma_start(out=bb_sb[:], in_=bb_d[:])
            iota_sb = cp.tile([P, P], fp16)
            nc.sync.dma_start(out=iota_sb[:], in_=iota_d[:])
            dinv_slot_sb = cp.tile([P, G], f32)
            nc.sync.dma_start(out=dinv_slot_sb[:], in_=dinv_slot_d[:])

            # Per-supergroup slices of the edge metadata get their own DMAs so
            # the first gather/S-build doesn't wait on the full upload.
            dstloc_sb = cp.tile([P, NT], fp16)
            dinv_sb = cp.tile([P, NT], f32)
            idx_sb = cp.tile([P, LTOT], i16)
            col0 = icol0 = 0
            meta_parts = []  # (col, icol, ncols) per supergroup
            for s in range(NSG):
                ncols = sg_lo[s] + sg_hi[s]
                nc.sync.dma_start(
                    out=dstloc_sb[:, col0 : col0 + ncols],
                    in_=dstloc_d[:, col0 : col0 + ncols],
                )
                nc.sync.dma_start(
                    out=dinv_sb[:, col0 : col0 + ncols],
                    in_=dinv_e_d[:, col0 : col0 + ncols],
                )
                nc.sync.dma_start(
                    out=idx_sb[:, icol0 : icol0 + ncols * 8],
                    in_=idx_d[:, icol0 : icol0 + ncols * 8],
                )
                meta_parts.append((col0, icol0, ncols))
                col0 += ncols
                icol0 += ncols * 8

            lo_tab = x_d[0:LO, :]
            hi_tab = x_d[LO:NPAD, :]

            qrr = 0
            icol = 0

            def gather(pool, tag, tab, nt):
                nonlocal icol, qrr
                m = pool.tile([P, nt * F], fp16, tag=tag)
                nc.gpsimd.dma_gather(
                    out_ap=m[:].rearrange("p (k f) -> p k f", f=F),
                    in_ap=tab,
                    idxs_ap=idx_sb[:, icol : icol + nt * 8],
                    num_idxs=nt * P,
                    num_idxs_reg=nt * P,
                    elem_size=F,
                    single_packet=False,
                    queue_num=qrr % 4,
                )
                qrr += 1
                icol += nt * 8
                return m

            # m-scale engine split: fraction R of tiles go to the (otherwise
            # idle) scalar engine as per-tile ACTIVATE Copy with per-partition
            # scale; the rest run on vector as one big tensor_tensor per
            # (slot, half) group.
            R_SCALAR = 0.70
            tiles_done = [0]
            tiles_scalar = [0]

            def scale_m(m, moff, nt, base_col):
                if nt == 0:
                    return
                use_scalar = tiles_scalar[0] < R_SCALAR * tiles_done[0]
                tiles_done[0] += nt
                if use_scalar:
                    tiles_scalar[0] += nt
                    for t in range(nt):
                        sl = m[:, (moff + t) * F : (moff + t + 1) * F]
                        nc.scalar.activation(
                            out=sl, in_=sl,
                            func=mybir.ActivationFunctionType.Copy,
                            scale=dinv_sb[:, base_col + t : base_col + t + 1],
                        )
                else:
                    v = m[:, moff * F : (moff + nt) * F]
                    nc.vector.tensor_tensor(
                        out=v.rearrange("p (t f) -> p t f", f=F),
                        in0=v.rearrange("p (t f) -> p t f", f=F),
                        in1=dinv_sb[:, base_col : base_col + nt].to_broadcast(
                            [P, nt, F]
                        ),
                        op=mybir.AluOpType.mult,
                    )

            col = 0
            for s in range(NSG):
                mlo = gather(plo, "mlo", lo_tab, sg_lo[s]) if sg_lo[s] else None
                mhi = gather(phi, "mhi", hi_tab, sg_hi[s]) if sg_hi[s] else None
                lo_col = col            # dstloc/dinv cols for this sg's lo run
                hi_col = col + sg_lo[s]
                mlo_off = 0
                mhi_off = 0
                for g in range(SPQ * s, SPQ * (s + 1)):
                    ntl, nth = T_LO[g], T_HI[g]
                    ntot = ntl + nth
                    agg = pagg.tile([P, P], f32, tag="agg")
                    mm = 0
                    for (nt, m, base_col, moff) in (
                        (ntl, mlo, lo_col, mlo_off),
                        (nth, mhi, hi_col, mhi_off),
                    ):
                        if nt == 0:
                            continue
                        scale_m(m, moff, nt, base_col)
                        S = psel.tile([P, nt * P], fp16, tag="S")
                        dl = dstloc_sb[:, base_col : base_col + nt]
                        nc.vector.tensor_tensor(
                            out=S[:].rearrange("p (t j) -> p t j", j=P),
                            in0=dl.to_broadcast([P, nt, P]),
                            in1=bass.AP(
                                iota_sb[:].tensor,
                                iota_sb[:].offset,
                                [iota_sb[:].ap[0], [0, nt], [1, P]],
                            ),
                            op=mybir.AluOpType.is_equal,
                        )
                        for t in range(nt):
                            nc.tensor.matmul(
                                out=agg[:],
                                lhsT=m[:, (moff + t) * F : (moff + t + 1) * F],
                                rhs=S[:, t * P : (t + 1) * P],
                                start=(mm == 0),
                                stop=(mm == ntot - 1),
                            )
                            mm += 1
                    lo_col += ntl
                    hi_col += nth
                    mlo_off += ntl
                    mhi_off += nth

                    tt = ptt.tile([P, P], f32, tag="tt")
                    nc.scalar.activation(
                        out=tt[:], in_=agg[:],
                        func=mybir.ActivationFunctionType.Copy,
                    )
                    gem = pgem.tile([P, P], f32, tag="gem")
                    nc.tensor.matmul(
                        out=gem[:], lhsT=tt[:], rhs=w_sb[:], start=True, stop=True
                    )
                    osb = posb.tile([P, P], f32, tag="osb")
                    nc.vector.tensor_scalar(
                        out=osb[:], in0=gem[:],
                        scalar1=dinv_slot_sb[:, g : g + 1], scalar2=None,
                        op0=mybir.AluOpType.mult,
                    )
                    nc.vector.tensor_tensor(
                        out=osb[:], in0=osb[:], in1=bb_sb[:],
                        op=mybir.AluOpType.add,
                    )
                    nc.sync.dma_start(
                        out=out_d[g * P : (g + 1) * P, :], in_=osb[:]
                    )
                col += sg_lo[s] + sg_hi[s]

    nc.compile()
    return nc


def _assemble(results):
    out = np.zeros((NB * P, F), np.float32)
    for c in range(NCORES):
        oc = results[c]["out"]
        for g in range(G):
            out[(8 * g + c) * P : (8 * g + c + 1) * P] = oc[g * P : (g + 1) * P]
    return out[:N]


def kernel(x, W, b, edge_index):
    from concourse.bass_utils import run_bass_kernel_spmd

    in_maps, T_LO, T_HI = _host_prep(x, W, b, edge_index)
    nc = build_nc(T_LO, T_HI)
    res = run_bass_kernel_spmd(nc, in_maps, list(range(NCORES)))
    return _assemble(res.results)


# revision 10
# speedup vs baseline: 1.6826x; 1.6826x over previous
"""GCN layer (gather + scatter-add message passing) on 8 Trainium2 NeuronCores.

Strategy (dst-partitioned node sharding, per the sharding hint):
  - Node blocks of 128; block b is owned by core b % 8, slot b // 8.
  - Host sorts edges by (dst block, src-table-half), appends self-loops, pads
    each (block, half) edge group to a multiple of 128, and precomputes the
    symmetric-normalization weights dinv = 1/sqrt(deg+1) from the integer
    degree counts (pure edge_index-derived metadata, like CSR setup).
  - Device: per (supergroup of 7 slots, table-half), one gpsimd dma_gather
    pulls the raw fp16 x rows for all edges (256B rows, software DGE).  Per
    128-edge tile the vector engine builds the scaled one-hot selection
    matrix S[e, n] = (dstloc[e] == n) * dinv_src[e] with a single fused
    tensor_scalar (is_equal + mult with two per-partition scalars - 4x DVE
    mode), and the tensor engine accumulates psum[f, n] += m[e, f]^T @ S[e, n].
    The dst-sorted layout means each 128-node block accumulates entirely in
    PSUM - no scatter to HBM at all.
  - Per block: out[n, o] = dinv[n] * (agg^T @ W)[n, o] + b[o], DMA'd straight
    to the core's output slice.  (The linear layer commutes with the
    aggregation, so the GEMM runs on the 6.3k aggregated rows per core
    instead of all 50k input rows.)

The edge tables are padded so the instruction stream is identical on all 8
cores (run_bass_kernel_spmd compiles one program); only tensor data differs.
"""

import sys

sys.path.insert(0, "/opt/trn_rl_repo")

import numpy as np

import concourse.bass as bass
import concourse.bacc as bacc
import concourse.mybir as mybir
import concourse.tile as tile
import concourse.tile_sem_assignment as _tsa
from concourse.tile import add_dep_helper

# Tile round-robins SWDGE DMAs over the 8 DMASW sem lanes in scheduling
# order, which lets one sem serve instructions on different SWDGE queues.
# The ucode's per-queue ring reclaim then sees foreign increments (CoreSim
# flags this as "sem locked to SWDGE queue").  Pin lanes per queue instead:
# queue q only ever uses lanes {q, q+4}.
if not getattr(_tsa.TileClockTick, "_gcn_queue_aware", False):
    _orig_assign_tick = _tsa.TileClockTick._assign_tick

    def _assign_tick_queue_aware(self, inst):
        if (
            isinstance(inst, _tsa.DMAInst)
            and inst.engine == mybir.EngineType.Pool
            and not isinstance(inst, _tsa.bass_isa.UserSyncedRemoteDMADescs)
            and self.swdge_sem_count == _tsa.NUM_SWDGE_GLOBAL_SEMS
        ):
            q = getattr(inst, "queue_num", 0) or 0
            toggles = getattr(self, "_gcn_q_toggle", None)
            if toggles is None:
                toggles = self._gcn_q_toggle = [0, 0, 0, 0]
            self.next_sw_dma_idx = q + 4 * toggles[q]
            toggles[q] ^= 1
        return _orig_assign_tick(self, inst)

    _tsa.TileClockTick._assign_tick = _assign_tick_queue_aware
    _tsa.TileClockTick._gcn_queue_aware = True

# Custom DVE op: scaled one-hot expansion for the aggregation matmul's rhs.
# out[p, k] = (k == dl[p, k]) ? dinv[p, k] : 0, with dl/dinv broadcast per
# 128-col page, so one big Vector instruction builds the selection matrices
# for a whole (slot, half) edge group (dl carries +128*page so the global
# free index Idx can be compared directly).
import concourse.dve_ops as _dve_ops
from concourse.dve_spec import Idx as _Idx, Spec as _Spec, Src0 as _Src0
from concourse.dve_spec import Src1 as _Src1, Zero as _Zero, eq as _eq
from concourse.dve_spec import select as _select


def _gcn_onehot_ref(in0, in1, s0, s1, imm2):
    p = in0.shape[0]
    a = in0.reshape(p, -1).astype(np.float32)
    b = in1.reshape(p, -1).astype(np.float32)
    k = np.arange(a.shape[1], dtype=np.float32)[None, :]
    return np.where(b == k, a, 0.0).reshape(in0.shape).astype(np.float32)


if "GCN_ONEHOT" not in _dve_ops._SUB_OPCODE_FOR_NAME:
    GCN_ONEHOT = _dve_ops.DveOp(
        "GCN_ONEHOT",
        _Spec(
            body=_select(_eq(_Idx, _Src1), _Src0, _Zero),
            reference=_gcn_onehot_ref,
        ),
        subdim=False,
        uops_sha={"v3": "d1e10c942dc632fb", "v4": "975c36a42d5c1811"},
    )
    _dve_ops.OPS.append(GCN_ONEHOT)
    _dve_ops.CUSTOM_DVE_SPECS[GCN_ONEHOT.name] = GCN_ONEHOT.spec
    _dve_ops._SUB_OPCODE_FOR_NAME[GCN_ONEHOT.name] = (
        _dve_ops._CUSTOM_DVE_ROW_BASE + len(_dve_ops.OPS) - 1
    )
else:
    GCN_ONEHOT = next(o for o in _dve_ops.OPS if o.name == "GCN_ONEHOT")

N = 50000
E = 800000
F = 128          # in/out channels
P = 128
NCORES = 8
NB = 392         # node blocks incl. padding (= 8 * 49)
G = NB // NCORES  # 49 slots per core
LO = 32768       # gather-table split (int16 index limit)
NPAD = 51200     # padded node rows
SPQ = 7          # slots per gather supergroup
NSG = G // SPQ   # 7 supergroups

f32 = mybir.dt.float32
fp16 = mybir.dt.float16
i32 = mybir.dt.int32
i16 = mybir.dt.int16


def _host_prep(x, W, b, edge_index):
    """Index manipulation + edge-metadata staging (no FP math on x/W)."""
    x = np.asarray(x, dtype=np.float32)
    W = np.asarray(W, dtype=np.float32)
    b = np.asarray(b, dtype=np.float32)
    ei = np.asarray(edge_index)
    src = ei[0].astype(np.int64)
    dst = ei[1].astype(np.int64)

    cnt = np.bincount(dst, minlength=NPAD).astype(np.int64)
    dinv = np.zeros(NPAD, np.float32)
    dinv[:N] = 1.0 / np.sqrt(cnt[:N].astype(np.float64) + 1.0)

    # Sort edges by (dst block, src table half).
    ishi = (src >= LO).astype(np.int64)
    blk = dst >> 7
    order = np.lexsort((ishi, blk))
    src_s, dst_s, ishi_s = src[order], dst[order], ishi[order]
    blk_s = blk[order]
    bounds = np.searchsorted(blk_s, np.arange(NB + 1))

    # Per (core, slot) edge lists.  block = 8*g + c.
    lo_idx = [[None] * G for _ in range(NCORES)]
    lo_dst = [[None] * G for _ in range(NCORES)]
    hi_idx = [[None] * G for _ in range(NCORES)]
    hi_dst = [[None] * G for _ in range(NCORES)]
    for g in range(G):
        for c in range(NCORES):
            bb_ = 8 * g + c
            s0, s1 = bounds[bb_], bounds[bb_ + 1]
            mid = s0 + int(np.searchsorted(ishi_s[s0:s1], 1))
            sl = np.arange(128 * bb_, min(128 * (bb_ + 1), N), dtype=np.int64)
            li, ld = src_s[s0:mid], dst_s[s0:mid] - 128 * bb_
            hi, hd = src_s[mid:s1] - LO, dst_s[mid:s1] - 128 * bb_
            if bb_ < LO // 128:  # self loops go in the lo half
                li = np.concatenate([li, sl])
                ld = np.concatenate([ld, sl - 128 * bb_])
            else:
                hi = np.concatenate([hi, sl - LO])
                hd = np.concatenate([hd, sl - 128 * bb_])
            lo_idx[c][g], lo_dst[c][g] = li, ld
            hi_idx[c][g], hi_dst[c][g] = hi, hd

    # Shared tile counts (max over cores) keep the instruction stream uniform.
    T_LO = [max(1, max(-(-len(lo_idx[c][g]) // 128) for c in range(NCORES)))
            for g in range(G)]
    T_HI = [max(-(-len(hi_idx[c][g]) // 128) for c in range(NCORES))
            for g in range(G)]
    NT = sum(T_LO) + sum(T_HI)
    LTOT = NT * 8

    x_pad = np.zeros((NPAD, F), np.float16)
    x_pad[:N] = x.astype(np.float16)
    bb_host = np.tile(b[None, :], (P, 1)).astype(np.float32)

    in_maps = []
    for c in range(NCORES):
        dstloc = np.full((P, NT), -1.0, np.float16)
        dinv_e = np.zeros((P, NT), np.float32)  # dinv[src] per edge
        idx16 = np.zeros((P, LTOT), np.int16)
        # Packing order mirrors the device issue order: per supergroup,
        # lo halves of its 7 slots, then hi halves of its 7 slots.
        col = icol = 0
        for sg in range(NSG):
            for side in ("lo", "hi"):
                for g in range(SPQ * sg, SPQ * (sg + 1)):
                    if side == "lo":
                        nt, li, ld = T_LO[g], lo_idx[c][g], lo_dst[c][g]
                        di = dinv[li]
                    else:
                        nt, li, ld = T_HI[g], hi_idx[c][g], hi_dst[c][g]
                        di = dinv[li + LO]
                    if nt == 0:
                        continue
                    assert nt <= 16, "dl_global must stay fp16-exact (< 2048)"
                    pi = np.zeros(nt * 128, np.int64)
                    pi[: len(li)] = li
                    # dl_global = local dst + 128 * (tile index within this
                    # slot-half's custom-DVE instruction); -1 marks padding.
                    pd = np.full(nt * 128, -1.0, np.float32)
                    pd[: len(ld)] = ld + 128.0 * (
                        np.arange(len(ld), dtype=np.float64) // 128
                    )
                    pv = np.zeros(nt * 128, np.float32)
                    pv[: len(li)] = di
                    dstloc[:, col : col + nt] = pd.reshape(nt, 128).T
                    dinv_e[:, col : col + nt] = pv.reshape(nt, 128).T
                    col += nt
                    k8 = nt * 8
                    idx16[:, icol : icol + k8] = np.tile(
                        pi.reshape(-1, 16).T.astype(np.int16), (8, 1)
                    )
                    icol += k8
        # dinv per owned node, laid out [128 node-in-block, G slots]
        dinv_slot = dinv.reshape(NPAD // P, P).T[:, [8 * g + c for g in range(G)]]
        in_maps.append(
            {
                "x": x_pad,
                "w": W,
                "bb": bb_host,
                "dstloc": dstloc,
                "dinv_e": dinv_e,
                "dinv_slot": np.ascontiguousarray(dinv_slot, np.float32),
                "idx16": idx16,
            }
        )
    return in_maps, T_LO, T_HI


def build_nc(T_LO, T_HI, debug=False):
    NT = sum(T_LO) + sum(T_HI)
    LTOT = NT * 8
    nc = bacc.Bacc(
        "TRN2", target_bir_lowering=False, debug=debug, num_swdge_queues=4
    )

    x_d = nc.dram_tensor("x", [NPAD, F], fp16, kind="ExternalInput")
    w_d = nc.dram_tensor("w", [F, F], f32, kind="ExternalInput")
    bb_d = nc.dram_tensor("bb", [P, F], f32, kind="ExternalInput")
    dstloc_d = nc.dram_tensor("dstloc", [P, NT], fp16, kind="ExternalInput")
    dinv_e_d = nc.dram_tensor("dinv_e", [P, NT], f32, kind="ExternalInput")
    dinv_slot_d = nc.dram_tensor("dinv_slot", [P, G], f32, kind="ExternalInput")
    idx_d = nc.dram_tensor("idx16", [P, LTOT], i16, kind="ExternalInput")
    out_d = nc.dram_tensor("out", [G * P, F], f32, kind="ExternalOutput")

    # Per-(supergroup, half) tile counts and column offsets.
    sg_lo = [sum(T_LO[SPQ * s : SPQ * (s + 1)]) for s in range(NSG)]
    sg_hi = [sum(T_HI[SPQ * s : SPQ * (s + 1)]) for s in range(NSG)]

    with tile.TileContext(nc) as tc:
        with (
            tc.tile_pool(name="const", bufs=1) as cp,
            tc.tile_pool(name="mlo", bufs=2) as plo,
            tc.tile_pool(name="mhi", bufs=2) as phi,
            tc.tile_pool(name="sel", bufs=6) as psel,
            tc.tile_pool(name="tt", bufs=3) as ptt,
            tc.tile_pool(name="osb", bufs=3) as posb,
            tc.tile_pool(name="agg", bufs=4, space="PSUM") as pagg,
            tc.tile_pool(name="gem", bufs=2, space="PSUM") as pgem,
        ):
            w_sb = cp.tile([F, F], f32)
            nc.sync.dma_start(out=w_sb[:], in_=w_d[:])
            bb_sb = cp.tile([P, F], f32)
            nc.sync.dma_start(out=bb_sb[:], in_=bb_d[:])
            dinv_slot_sb = cp.tile([P, G], f32)
            nc.sync.dma_start(out=dinv_slot_sb[:], in_=dinv_slot_d[:])

            # Per-supergroup slices of the edge metadata get their own DMAs so
            # the first gather/S-build doesn't wait on the full upload.
            dstloc_sb = cp.tile([P, NT], fp16)
            dinv_sb = cp.tile([P, NT], f32)
            idx_sb = cp.tile([P, LTOT], i16)
            col0 = icol0 = 0
            meta_parts = []  # (col, icol, ncols) per supergroup
            for s in range(NSG):
                ncols = sg_lo[s] + sg_hi[s]
                nc.sync.dma_start(
                    out=dstloc_sb[:, col0 : col0 + ncols],
                    in_=dstloc_d[:, col0 : col0 + ncols],
                )
                nc.sync.dma_start(
                    out=dinv_sb[:, col0 : col0 + ncols],
                    in_=dinv_e_d[:, col0 : col0 + ncols],
                )
                nc.sync.dma_start(
                    out=idx_sb[:, icol0 : icol0 + ncols * 8],
                    in_=idx_d[:, icol0 : icol0 + ncols * 8],
                )
                meta_parts.append((col0, icol0, ncols))
                col0 += ncols
                icol0 += ncols * 8

            lo_tab = x_d[0:LO, :]
            hi_tab = x_d[LO:NPAD, :]

            qrr = 0
            icol = 0

            def gather(pool, tag, tab, nt):
                nonlocal icol, qrr
                m = pool.tile([P, nt * F], fp16, tag=tag)
                nc.gpsimd.dma_gather(
                    out_ap=m[:].rearrange("p (k f) -> p k f", f=F),
                    in_ap=tab,
                    idxs_ap=idx_sb[:, icol : icol + nt * 8],
                    num_idxs=nt * P,
                    num_idxs_reg=nt * P,
                    elem_size=F,
                    single_packet=False,
                    queue_num=qrr % 4,
                )
                qrr += 1
                icol += nt * 8
                return m

            col = 0
            for s in range(NSG):
                mlo = gather(plo, "mlo", lo_tab, sg_lo[s]) if sg_lo[s] else None
                mhi = gather(phi, "mhi", hi_tab, sg_hi[s]) if sg_hi[s] else None
                lo_col = col            # dstloc/dinv cols for this sg's lo run
                hi_col = col + sg_lo[s]
                mlo_off = 0
                mhi_off = 0
                for g in range(SPQ * s, SPQ * (s + 1)):
                    ntl, nth = T_LO[g], T_HI[g]
                    ntot = ntl + nth
                    agg = pagg.tile([P, P], f32, tag="agg")
                    mm = 0
                    for (nt, m, base_col, moff) in (
                        (ntl, mlo, lo_col, mlo_off),
                        (nth, mhi, hi_col, mhi_off),
                    ):
                        if nt == 0:
                            continue
                        # One custom-DVE instruction builds the scaled one-hot
                        # rhs for all nt tiles of this (slot, half) group:
                        # S[e, t*128+j] = (dl_global[e,t] == t*128+j) * dinv[e,t]
                        S = psel.tile([P, nt * P], fp16, tag="S")
                        nc.vector._custom_dve(
                            GCN_ONEHOT,
                            out=S[:].rearrange("p (t j) -> p t j", j=P),
                            in0=dinv_sb[:, base_col : base_col + nt].to_broadcast(
                                [P, nt, P]
                            ),
                            in1=dstloc_sb[
                                :, base_col : base_col + nt
                            ].to_broadcast([P, nt, P]),
                        )
                        for t in range(nt):
                            nc.tensor.matmul(
                                out=agg[:],
                                lhsT=m[:, (moff + t) * F : (moff + t + 1) * F],
                                rhs=S[:, t * P : (t + 1) * P],
                                start=(mm == 0),
                                stop=(mm == ntot - 1),
                            )
                            mm += 1
                    lo_col += ntl
                    hi_col += nth
                    mlo_off += ntl
                    mhi_off += nth

                    tt = ptt.tile([P, P], f32, tag="tt")
                    nc.scalar.activation(
                        out=tt[:], in_=agg[:],
                        func=mybir.ActivationFunctionType.Copy,
                    )
                    gem = pgem.tile([P, P], f32, tag="gem")
                    nc.tensor.matmul(
                        out=gem[:], lhsT=tt[:], rhs=w_sb[:], start=True, stop=True
                    )
                    osb = posb.tile([P, P], f32, tag="osb")
                    nc.vector.tensor_scalar(
                        out=osb[:], in0=gem[:],
                        scalar1=dinv_slot_sb[:, g : g + 1], scalar2=None,
                        op0=mybir.AluOpType.mult,
                    )
                    nc.vector.tensor_tensor(
                        out=osb[:], in0=osb[:], in1=bb_sb[:],
                        op=mybir.AluOpType.add,
                    )
                    nc.sync.dma_start(
                        out=out_d[g * P : (g + 1) * P, :], in_=osb[:]
                    )
                col += sg_lo[s] + sg_hi[s]

    nc.compile()
    return nc


def _assemble(results):
    out = np.zeros((NB * P, F), np.float32)
    for c in range(NCORES):
        oc = results[c]["out"]
        for g in range(G):
            out[(8 * g + c) * P : (8 * g + c + 1) * P] = oc[g * P : (g + 1) * P]
    return out[:N]


def kernel(x, W, b, edge_index):
    from concourse.bass_utils import run_bass_kernel_spmd

    in_maps, T_LO, T_HI = _host_prep(x, W, b, edge_index)
    nc = build_nc(T_LO, T_HI)
    res = run_bass_kernel_spmd(nc, in_maps, list(range(NCORES)))
    return _assemble(res.results)
